# revision 7
# baseline (speedup 1.0000x reference)
"""Trainium2 Bass kernel for nn_AtBatCell: GRU recurrence over a shared state
table with gather/scatter-add per timestep.

Strategy: steps that touch disjoint table rows are independent, so the T=8192
sequential scan collapses into a few "waves" (levels of the row-dependency
DAG; ~6 for this input distribution). Each wave is a large batch of
independent GRU cell applications: dma_gather rows -> batched matmuls
(float32r on the PE) -> elementwise -> dma_scatter_add deltas back.
Host does scheduling/index prep only; all FLOPs run on device.
"""
import sys
sys.path.insert(0, '/opt/trn_rl_repo')

import numpy as np

SIT = 64
S = 256
S2 = 512
CHUNK = 128          # steps per compute chunk
SPARE = 128          # spare zero rows absorbing padding scatters
SCATTER_BATCH = 4    # chunks per scatter-add call


def _schedule(b, p, n_rows_total):
    """Wave levels + compacted row ids. Returns schedule dict."""
    T = len(b)
    last_level = np.zeros(n_rows_total, dtype=np.int64)
    levels = np.empty(T, dtype=np.int64)
    bl = b.astype(np.int64)
    pl = p.astype(np.int64)
    for t in range(T):
        lv = max(last_level[bl[t]], last_level[pl[t]]) + 1
        levels[t] = lv
        last_level[bl[t]] = lv
        last_level[pl[t]] = lv
    n_waves = int(levels.max())
    order = np.argsort(levels, kind='stable')
    wave_sizes = np.bincount(levels, minlength=n_waves + 1)[1:]

    touched = np.unique(np.concatenate([bl, pl]))
    remap = np.full(n_rows_total, -1, dtype=np.int64)
    remap[touched] = np.arange(len(touched))
    return dict(levels=levels, order=order, wave_sizes=wave_sizes,
                n_waves=n_waves, touched=touched, remap=remap)


def _build_host_data(x, b, p, Wz, Wr, Wh, Uz, Ur, Uh, bz, br, bh, table0):
    N = table0.shape[0]
    sch = _schedule(b, p, N)
    order, wave_sizes = sch['order'], sch['wave_sizes']
    touched, remap = sch['touched'], sch['remap']
    n_real = len(touched)
    n_rows_c = n_real + SPARE

    # per-wave chunking with padding to CHUNK
    wave_chunks = [int(-(-int(ws) // CHUNK)) for ws in wave_sizes]
    n_chunks = sum(wave_chunks)
    T_pad = n_chunks * CHUNK

    # schedule-ordered step arrays, padded
    b_s = np.full(T_pad, -1, dtype=np.int64)
    p_s = np.full(T_pad, -1, dtype=np.int64)
    x_s = np.zeros((T_pad, SIT), dtype=np.float32)
    bias_col = np.zeros(T_pad, dtype=np.float32)
    pos = 0
    src = 0
    dup_any = False
    for w, ws in enumerate(wave_sizes):
        ws = int(ws)
        idxs = order[src:src + ws]
        src += ws
        b_w = remap[b[idxs].astype(np.int64)]
        p_w = remap[p[idxs].astype(np.int64)]
        # dup handling: if b==p within a step, redirect p-slot to a spare row
        dup = (b_w == p_w)
        if dup.any():
            dup_any = True
        b_s[pos:pos + ws] = b_w
        p_s[pos:pos + ws] = p_w
        x_s[pos:pos + ws] = x[idxs]
        bias_col[pos:pos + ws] = 1.0
        pos += -(-ws // CHUNK) * CHUNK

    # padding / dup slots -> distinct spare rows (cycled per chunk)
    spare_ids = n_real + np.arange(SPARE)
    pad_mask = b_s < 0
    # each pad step gets spare row (i % SPARE); unique within a chunk & wave
    pad_pos = np.nonzero(pad_mask)[0]
    b_s[pad_pos] = spare_ids[pad_pos % SPARE]
    p_s[pad_pos] = spare_ids[(pad_pos + 1) % SPARE]  # ok: dh==0 exactly for pads

    dup_steps = np.nonzero(b_s == p_s)[0]
    dupmask = np.zeros(T_pad, dtype=np.float32)
    if len(dup_steps):
        dupmask[dup_steps] = 1.0
        p_s[dup_steps] = spare_ids[dup_steps % SPARE]

    # interleaved idx order: per chunk [b_0..127 | p_0..127]
    idx_il = np.empty(2 * T_pad, dtype=np.int16)
    bi = b_s.reshape(n_chunks, CHUNK)
    pi = p_s.reshape(n_chunks, CHUNK)
    idx_il = np.concatenate([np.stack([bi, pi], axis=1).reshape(-1)]).astype(np.int16)
    # wrapped layout [16, n/16] then replicated to 128 partitions
    n_idx = 2 * T_pad
    idx_wrapped = idx_il.reshape(n_idx // 16, 16).T  # [16, n/16]
    idx_rep = np.tile(idx_wrapped, (8, 1)).copy()    # [128, n/16]

    # xT augmented with bias row (65, T_pad)
    xT = np.zeros((SIT + 1, T_pad), dtype=np.float32)
    xT[:SIT] = x_s.T
    xT[SIT] = bias_col

    # weight tensors, host-transposed
    WzT = np.concatenate([Wz.T, bz[None, :]], axis=0).astype(np.float32)   # (65,512)
    WrT = np.concatenate([Wr.T, -br[None, :]], axis=0).astype(np.float32)
    WhT = np.concatenate([Wh.T, bh[None, :]], axis=0).astype(np.float32)
    # UT[k, n] = U[n, k] reshaped to [128, 4, 512]
    def ut(U):
        return np.ascontiguousarray(
            U.T.reshape(4, 128, S2).transpose(1, 0, 2)).astype(np.float32)
    UzT, UrT, UhT = ut(Uz), ut(Ur), ut(Uh)

    table_c = np.zeros((n_rows_c, S), dtype=np.float32)
    table_c[:n_real] = table0[touched]

    chunk_wave = []
    for w, wc in enumerate(wave_chunks):
        chunk_wave += [w] * wc

    return dict(
        table_c=table_c, idx_rep=idx_rep, xT=xT,
        WzT=WzT, WrT=WrT, WhT=WhT, UzT=UzT, UrT=UrT, UhT=UhT,
        n_chunks=n_chunks, wave_chunks=wave_chunks, chunk_wave=chunk_wave,
        n_rows_c=n_rows_c, n_real=n_real, touched=touched,
        dupmask=dupmask, dup_any=(len(dup_steps) > 0), T_pad=T_pad,
    )


def _build_nc(hd, dbg_max_chunks=None, dbg_no_scatter=False, dbg_no_copy=False):
    import os
    import concourse.bacc as bacc
    import concourse.mybir as mybir
    import concourse.tile as tile
    from concourse.masks import make_identity
    if dbg_max_chunks is None:
        dbg_max_chunks = int(os.environ.get("DBG_MAX_CHUNKS", "0")) or None
    dbg_no_scatter = dbg_no_scatter or os.environ.get("DBG_NO_SCATTER") == "1"
    dbg_no_copy = dbg_no_copy or os.environ.get("DBG_NO_COPY") == "1"

    n_rows_c = hd['n_rows_c']
    n_chunks = hd['n_chunks']
    T_pad = hd['T_pad']
    wave_chunks = hd['wave_chunks']
    f32 = mybir.dt.float32
    f32r = mybir.dt.float32r
    i16 = mybir.dt.int16

    nc = bacc.Bacc("TRN2", target_bir_lowering=False, debug=True)

    tab_in = nc.dram_tensor("table", (n_rows_c, S), f32, kind="ExternalInput")
    idx_in = nc.dram_tensor("idx", (128, 2 * T_pad // 16), i16, kind="ExternalInput")
    xT_in = nc.dram_tensor("xT", (SIT + 1, T_pad), f32r, kind="ExternalInput")
    WzT_in = nc.dram_tensor("WzT", (SIT + 1, S2), f32r, kind="ExternalInput")
    WrT_in = nc.dram_tensor("WrT", (SIT + 1, S2), f32r, kind="ExternalInput")
    WhT_in = nc.dram_tensor("WhT", (SIT + 1, S2), f32r, kind="ExternalInput")
    UzT_in = nc.dram_tensor("UzT", (128, 4, S2), f32r, kind="ExternalInput")
    UrT_in = nc.dram_tensor("UrT", (128, 4, S2), f32r, kind="ExternalInput")
    UhT_in = nc.dram_tensor("UhT", (128, 4, S2), f32r, kind="ExternalInput")
    dmask_in = nc.dram_tensor("dmask", (128, n_chunks), f32, kind="ExternalInput")

    tab_out = nc.dram_tensor("out", (n_rows_c, S), f32, kind="ExternalOutput")

    with tile.TileContext(nc) as tc:
        with tc.tile_pool(name="const", bufs=1) as cpool, \
             tc.tile_pool(name="gath", bufs=18) as gpool, \
             tc.tile_pool(name="dhb", bufs=2) as dhpool, \
             tc.tile_pool(name="work", bufs=2) as wpool, \
             tc.tile_pool(name="psA", bufs=2, space="PSUM") as psA, \
             tc.tile_pool(name="psZ", bufs=2, space="PSUM") as psZ, \
             tc.tile_pool(name="psR", bufs=2, space="PSUM") as psR, \
             tc.tile_pool(name="psM", bufs=2, space="PSUM") as psM:

            # ---- static loads ----
            if not dbg_no_copy:
                # init copy, sliced: a single 13MB D2D DMA faults the engine
                CP = 1024  # rows per slice (1MB)
                for r0 in range(0, n_rows_c, CP):
                    r1 = min(r0 + CP, n_rows_c)
                    nc.sync.dma_start(tab_out[r0:r1, :], tab_in[r0:r1, :])

            idx_sb = cpool.tile([128, 2 * T_pad // 16], i16, tag="idx")
            nc.sync.dma_start(idx_sb[:], idx_in[:])
            xT_sb = cpool.tile([SIT + 1, T_pad], f32r, tag="xT")
            nc.sync.dma_start(xT_sb[:], xT_in[:])
            w_sb = {}
            for nm, t in (("WzT", WzT_in), ("WrT", WrT_in), ("WhT", WhT_in)):
                w_sb[nm] = cpool.tile([SIT + 1, S2], f32r, tag=nm, name=nm + "_sb")
                nc.sync.dma_start(w_sb[nm][:], t[:])
            for nm, t in (("UzT", UzT_in), ("UrT", UrT_in), ("UhT", UhT_in)):
                w_sb[nm] = cpool.tile([128, 4, S2], f32r, tag=nm, name=nm + "_sb")
                nc.sync.dma_start(w_sb[nm][:], t[:])
            dmask_sb = cpool.tile([128, n_chunks], f32, tag="dmask")
            if hd['dup_any']:
                nc.sync.dma_start(dmask_sb[:], dmask_in[:])
            ident = cpool.tile([128, 128], f32, tag="ident")
            make_identity(nc, ident[:])

            Sig = mybir.ActivationFunctionType.Sigmoid
            Tanh = mybir.ActivationFunctionType.Tanh

            chunk0 = 0
            emitted = 0
            for w, wc in enumerate(wave_chunks):
                if dbg_max_chunks is not None and emitted >= dbg_max_chunks:
                    break
                if dbg_max_chunks is not None:
                    wc = min(wc, dbg_max_chunks - emitted)
                emitted += wc
                src_tab = tab_in if w == 0 else tab_out
                hg_tiles = []
                # -- gathers (reads) first --
                for ci in range(wc):
                    c = chunk0 + ci
                    hg = gpool.tile([128, 2, S], f32, tag="hg")
                    nc.gpsimd.dma_gather(
                        out_ap=hg[:], in_ap=src_tab[:],
                        idxs_ap=idx_sb[:, 16 * c:16 * (c + 1)],
                        num_idxs=2 * CHUNK, num_idxs_reg=2 * CHUNK,
                        elem_size=S,
                    )
                    hg_tiles.append(hg)
                # -- compute per chunk; dh written into batch tiles --
                dh_tiles = {}
                for ci in range(wc):
                    c = chunk0 + ci
                    bi = ci // SCATTER_BATCH
                    q = ci % SCATTER_BATCH
                    if q == 0:
                        nb = min(SCATTER_BATCH, wc - ci)
                        dh_tiles[bi] = (dhpool.tile(
                            [128, 2 * SCATTER_BATCH, S], f32, tag="dh",
                            name=f"dh_{w}_{bi}"), nb)
                    dhb = dh_tiles[bi][0]

                    hg = hg_tiles[ci]
                    hg2 = hg[:].rearrange("p a b -> p (a b)")

                    # transpose H -> HT (4x PE transpose, 1 PSUM tile)
                    ht_ps = psA.tile([128, 4, 128], f32, tag="tr")
                    for k in range(4):
                        nc.tensor.transpose(
                            ht_ps[:, k, :], hg2[:, 128 * k:128 * (k + 1)], ident[:])
                    ht = wpool.tile([128, 4, 128], f32r, tag="ht")
                    nc.vector.tensor_copy(ht[:], ht_ps[:])

                    xt_c = xT_sb[:, CHUNK * c:CHUNK * (c + 1)]

                    zpre = psZ.tile([128, S2], f32, tag="zpre")
                    rpre = psR.tile([128, S2], f32, tag="rpre")
                    nc.tensor.matmul(zpre[:], xt_c, w_sb["WzT"][:], start=True, stop=False)
                    for k in range(4):
                        nc.tensor.matmul(zpre[:], ht[:, k, :], w_sb["UzT"][:, k, :],
                                         start=False, stop=(k == 3))
                    nc.tensor.matmul(rpre[:], xt_c, w_sb["WrT"][:], start=True, stop=False)
                    for k in range(4):
                        nc.tensor.matmul(rpre[:], ht[:, k, :], w_sb["UrT"][:, k, :],
                                         start=False, stop=(k == 3))

                    z = wpool.tile([128, S2], f32, tag="z")
                    r = wpool.tile([128, S2], f32, tag="r")
                    nc.scalar.activation(z[:], zpre[:], Sig)
                    nc.scalar.activation(r[:], rpre[:], Sig)

                    rh = wpool.tile([128, S2], f32, tag="rh")
                    nc.vector.tensor_mul(rh[:], r[:], hg2)

                    rht_ps = psA.tile([128, 4, 128], f32, tag="tr")
                    for k in range(4):
                        nc.tensor.transpose(
                            rht_ps[:, k, :], rh[:, 128 * k:128 * (k + 1)], ident[:])
                    rht = wpool.tile([128, 4, 128], f32r, tag="rht")
                    nc.vector.tensor_copy(rht[:], rht_ps[:])

                    mpre = psM.tile([128, S2], f32, tag="mpre")
                    nc.tensor.matmul(mpre[:], xt_c, w_sb["WhT"][:], start=True, stop=False)
                    for k in range(4):
                        nc.tensor.matmul(mpre[:], rht[:, k, :], w_sb["UhT"][:, k, :],
                                         start=False, stop=(k == 3))

                    m = wpool.tile([128, S2], f32, tag="m")
                    nc.scalar.activation(m[:], mpre[:], Tanh)

                    # dh = (1-z)*(m-h) = t1 - z*t1
                    t1 = wpool.tile([128, S2], f32, tag="t1")
                    nc.vector.tensor_sub(t1[:], m[:], hg2)
                    t2 = wpool.tile([128, S2], f32, tag="t2")
                    nc.vector.tensor_mul(t2[:], z[:], t1[:])
                    dh_view = dhb[:, 2 * q:2 * (q + 1), :].rearrange("p a b -> p (a b)")
                    nc.vector.tensor_sub(dh_view, t1[:], t2[:])
                    if hd['dup_any']:
                        # fold p-half delta into b-half for steps with b==p
                        tm = wpool.tile([128, S], f32, tag="tm")
                        nc.vector.tensor_scalar_mul(
                            tm[:], dhb[:, 2 * q + 1, :], dmask_sb[:, c:c + 1])
                        nc.vector.tensor_add(
                            dhb[:, 2 * q, :], dhb[:, 2 * q, :], tm[:])

                    # flush scatter batch when full / wave end
                    nb = dh_tiles[bi][1]
                    if q == nb - 1 and not dbg_no_scatter:
                        c0 = chunk0 + bi * SCATTER_BATCH
                        nidx = 2 * CHUNK * nb
                        nc.gpsimd.dma_scatter_add(
                            tab_out[:],
                            dhb[:, 0:2 * nb, :],
                            idx_sb[:, 16 * c0:16 * c0 + nidx // 16],
                            nidx, nidx, S,
                        )
                chunk0 += wc

    nc.compile()
    return nc


_CACHE = {}


def kernel(**inputs):
    x = np.asarray(inputs['x'], dtype=np.float32)
    b = np.asarray(inputs['b'])
    p = np.asarray(inputs['p'])
    table0 = np.asarray(inputs['table0'], dtype=np.float32)

    hd = _build_host_data(
        x, b, p,
        np.asarray(inputs['Wz'], np.float32), np.asarray(inputs['Wr'], np.float32),
        np.asarray(inputs['Wh'], np.float32), np.asarray(inputs['Uz'], np.float32),
        np.asarray(inputs['Ur'], np.float32), np.asarray(inputs['Uh'], np.float32),
        np.asarray(inputs['bz'], np.float32), np.asarray(inputs['br'], np.float32),
        np.asarray(inputs['bh'], np.float32), table0)

    from concourse.bass_utils import run_bass_kernel_spmd

    nc = _build_nc(hd)

    dmask_full = np.zeros((128, hd['n_chunks']), dtype=np.float32)
    dm = hd['dupmask'].reshape(hd['n_chunks'], CHUNK).T  # [128, n_chunks]
    dmask_full[:] = dm

    in_map = {
        "table": hd['table_c'], "idx": hd['idx_rep'], "xT": hd['xT'],
        "WzT": hd['WzT'], "WrT": hd['WrT'], "WhT": hd['WhT'],
        "UzT": hd['UzT'], "UrT": hd['UrT'], "UhT": hd['UhT'],
        "dmask": dmask_full,
    }
    res = run_bass_kernel_spmd(nc, [dict(in_map) for _ in range(8)],
                               list(range(8)))
    tab_res = res.results[0]["out"]

    out = table0.copy()
    out[hd['touched']] = tab_res[:hd['n_real']]
    return out


if __name__ == "__main__":
    import reference
    inputs = {k: np.asarray(v) for k, v in reference.setup_inputs().items()}
    got = kernel(**inputs)
    exp = np.asarray(reference.reference(**inputs))
    err = np.abs(got - exp).max()
    scale = np.abs(exp).max()
    print("abs err:", err, "rel:", err / scale)
